# revision 1
# baseline (speedup 1.0000x reference)
"""LocalFrameAttention TRN2 kernel.

Problem: x[B=2,F=16,N=256,D=1024] -> qkv proj -> chunked local attention
(chunk = 4 frames = 1024 tokens; chunk c attends to chunks {c-1, c}, chunk 0
to itself) -> out proj.  H=16 heads, HD=64.

Sharding: 8 cores = B(2) x head-groups(4).  Each core handles 4 heads for all
16 frames of one batch: column-parallel qkv projection, full SDPA for its
heads, row-parallel out projection producing a partial [D, T] output; the
host sums the 4 partials per batch (tensor-parallel reduce) and transposes.

Layouts (on-chip activations kept "transposed", d-major):
  - XT [D, T] host-transposed; K^T/Q^T [e=256, T] via lhsT=W^T, rhs=XT;
  - V [T, e] via lhsT=XT, rhs=Wv^T, stored [128, tok_tile, head, 65] with a
    ones 65th column (softmax denominator via the PV matmul);
  - S^T [k_tok=128, q 512 x 2 heads] = mm(lhsT=K^T, rhs=Q^T), K=64
    contraction, two heads row-paired on the PE;
  - P^T = exp(S^T) on ACT (no max subtraction; |logits| small), fp16;
  - O^T accum [65, 512] = mm(lhsT=[V|1], rhs=P^T) over window k-tiles;
  - normalize: O^T copied out of PSUM immediately (frees accumulator banks),
    reciprocal of row 64, partition-broadcast via a DRAM round-trip DMA,
    DVE muls;
  - out proj: partial^T [dd, tok] = mm(lhsT=Wo^T, rhs=O^T).

All matmul operands are fp16 (fp32 PSUM accumulation).  fp16 gets separate,
pipelined LDWEIGHTS (4-byte dtypes force a self-loading matmul that
serializes the weight load) at 2x the mantissa error of fp32r (2^-11).
Next-chunk projection and prev-chunk out-projection matmul groups are
interleaved into the SDPA loop as stall filler (engine instruction order is
static on TRN2).

Measured: ~455 us/core for the full pass (all 8 cores run in parallel),
max relative error vs the fp32 reference ~5e-4.
"""

import sys

if "/opt/trn_rl_repo" not in sys.path:
    sys.path.insert(0, "/opt/trn_rl_repo")

import numpy as np

import concourse.bass as bass  # noqa: F401
import concourse.mybir as mybir
import concourse.tile as tile
from concourse import bacc
from concourse.bass_utils import run_bass_kernel_spmd

F32 = mybir.dt.float32
F32R = mybir.dt.float32r
F16 = mybir.dt.float16
EXP = mybir.ActivationFunctionType.Exp

B, F, N, D = 2, 16, 256, 1024
H, HD, CHUNK = 16, 64, 4
C = F // CHUNK            # 4 chunks
CT = CHUNK * N            # 1024 tokens per chunk
T = F * N                 # 4096 tokens per batch
HL = 4                    # heads per core
E = HL * HD               # 256 local qkv width
NCORES = 8

_cached = {}


def _round_fp32r(a: np.ndarray) -> np.ndarray:
    """Round fp32 array to fp32r (11-bit mantissa, value in high 20 bits)."""
    u = np.ascontiguousarray(a, dtype=np.float32).view(np.uint32)
    r = ((u.astype(np.uint64) + 0x800) & 0xFFFFF000).astype(np.uint32)
    return r.view(np.float32)


def _emit_pass(nc, pools, aps, stage):
    """One full compute pass, software-pipelined: next-chunk projection and
    previous-chunk out-projection matmul groups are interleaved into the SDPA
    kt loop so the PE's static instruction order has independent filler work
    at every exp-dependency stall point."""
    (xpool, kpool, qpool, vpool, ppool, opool, spool, fpool, dpool,
     ps_s, ps_o, ps_m) = pools
    xt_d, pt_d, wkq_sb, wv_sb, wo_sb, ones_sb = aps

    kt_ring = {}
    qt_ring = {}
    v_ring = {}
    xt_tiles = {}
    ot_ring = {}

    def proj_closures(c):
        """19 closures: tile allocs + per-token-block DMA + 8 matmul groups."""
        cl = []

        def alloc(c=c):
            kt_ring[c] = kpool.tile([128, 2, CT], F16, tag="kt", name=f"kt{c}")
            qt_ring[c] = qpool.tile([128, 2, CT], F16, tag="qt", name=f"qt{c}")
            v_c = vpool.tile([128, 8, HL, 68], F16, tag="v")
            nc.vector.memset(v_c[:], 1.0)  # ones col; rest overwritten
            v_ring[c] = v_c

        cl.append(alloc)
        for tb in range(2):
            def dma(c=c, tb=tb):
                xt_t = xpool.tile([128, 8, 512], F16, tag="xt")
                t0 = c * CT + tb * 512
                nc.sync.dma_start(
                    xt_t[:],
                    xt_d[:, t0 : t0 + 512].rearrange("(dt p) t -> p dt t", p=128),
                )
                xt_tiles[(c, tb)] = xt_t

            cl.append(dma)
            for et in range(4):  # K^T (0,1), Q^T (2,3)
                def kq_group(c=c, tb=tb, et=et):
                    xt_t = xt_tiles[(c, tb)]
                    ps = ps_m.tile([128, 512], F32, tag="m")
                    for dt in range(8):
                        nc.tensor.matmul(
                            ps[:],
                            wkq_sb[:, dt, et * 128 : (et + 1) * 128],
                            xt_t[:, dt, :],
                            start=(dt == 0),
                            stop=(dt == 7),
                        )
                    dst = kt_ring[c] if et < 2 else qt_ring[c]
                    nc.vector.tensor_copy(
                        dst[:, et % 2, tb * 512 : (tb + 1) * 512], ps[:]
                    )

                cl.append(kq_group)
            for tt in range(4):  # V tok-tiles
                def v_group(c=c, tb=tb, tt=tt):
                    xt_t = xt_tiles[(c, tb)]
                    ps = ps_m.tile([128, 512], F32, tag="m")
                    for dt in range(8):
                        nc.tensor.matmul(
                            ps[:, 0:E],
                            xt_t[:, dt, tt * 128 : (tt + 1) * 128],
                            wv_sb[:, dt, :],
                            start=(dt == 0),
                            stop=(dt == 7),
                        )
                    nc.vector.tensor_copy(
                        v_ring[c][:, tb * 4 + tt, :, 0:64],
                        ps[:, 0:E].rearrange("p (h d) -> p h d", h=HL),
                    )

                cl.append(v_group)
        return cl

    def outproj_closures(c):
        cl = []
        for ddt in range(8):
            for tb in range(2):
                def op_group(c=c, ddt=ddt, tb=tb):
                    ot_c = ot_ring[c]
                    fp = ps_m.tile([128, 512], F32, tag="m")
                    for et in range(2):
                        nc.tensor.matmul(
                            fp[:],
                            wo_sb[:, et, ddt * 128 : (ddt + 1) * 128],
                            ot_c[:, et, tb * 512 : (tb + 1) * 512],
                            start=(et == 0),
                            stop=(et == 1),
                        )
                    fin = fpool.tile([128, 512], F32, tag="fin")
                    nc.vector.tensor_copy(fin[:], fp[:])
                    nc.sync.dma_start(
                        pt_d[
                            ddt * 128 : (ddt + 1) * 128,
                            c * CT + tb * 512 : c * CT + (tb + 1) * 512,
                        ],
                        fin[:],
                    )

                cl.append(op_group)
        return cl

    def sdpa_chunk(c, filler):
        """Emit SDPA(c); pop one filler closure after every few kt steps."""
        kts = (
            [(c, i) for i in range(8)]
            if c == 0
            else [(c - 1, i) for i in range(8)] + [(c, i) for i in range(8)]
        )
        ot_ring[c] = opool.tile([128, 2, CT], F16, tag="ot", name=f"ot{c}")
        ot_c = ot_ring[c]
        qt_c = qt_ring[c]

        n_steps = 4 * len(kts)
        stride = max(1, -(-n_steps // max(1, len(filler))) )
        step = 0

        if stage == "proj":
            nc.vector.tensor_copy(ot_c[:, :, 0:4], kt_ring[c][:, :, 0:4])
            while filler:
                filler.pop(0)()
            return

        for hp in range(2):
            for qb in range(2):
                o0 = ps_o.tile([65, 512], F32, tag="o")
                o1 = ps_o.tile([65, 512], F32, tag="o")
                last = len(kts) - 1
                for i, (kc, kt) in enumerate(kts):
                    s = ps_s.tile([128, 1024], F32, tag="s")
                    ktile = kt_ring[kc]
                    for hl in range(2):
                        r0, r1 = hl * 64, hl * 64 + 64
                        nc.tensor.matmul(
                            s[:, hl * 512 : hl * 512 + 512],
                            ktile[r0:r1, hp, kt * 128 : (kt + 1) * 128],
                            qt_c[r0:r1, hp, qb * 512 : (qb + 1) * 512],
                            start=True,
                            stop=True,
                        )
                    p = ppool.tile([128, 1024], F16, tag="p")
                    if stage in ("qkexp", "full"):
                        nc.scalar.activation(p[:], s[:], EXP)
                    else:
                        nc.vector.tensor_copy(p[:, 0:4], s[:, 0:4])
                    if stage == "full":
                        vt = v_ring[kc]
                        nc.tensor.matmul(
                            o0[:],
                            vt[:, kt, 2 * hp, 0:65],
                            p[:, 0:512],
                            start=(i == 0),
                            stop=(i == last),
                        )
                        nc.tensor.matmul(
                            o1[:],
                            vt[:, kt, 2 * hp + 1, 0:65],
                            p[:, 512:1024],
                            start=(i == 0),
                            stop=(i == last),
                        )
                    elif i == 0:
                        nc.vector.tensor_copy(o0[:, 0:4], p[0:65, 0:4])
                        nc.vector.tensor_copy(o1[:, 0:4], p[0:65, 0:4])
                    step += 1
                    if filler and step % stride == 0:
                        filler.pop(0)()
                # stage O^T out of PSUM immediately (frees the accumulator
                # banks for the next block's PV), then normalize from SBUF
                osb = spool.tile([65, 1024], F32, tag="osb")
                nc.vector.tensor_copy(osb[:, 0:512], o0[:])
                nc.vector.tensor_copy(osb[:, 512:1024], o1[:])
                rd0 = spool.tile([1, 512], F32, tag="rd0")
                rd1 = spool.tile([1, 512], F32, tag="rd1")
                nc.vector.reciprocal(rd0[:], osb[64:65, 0:512])
                nc.vector.reciprocal(rd1[:], osb[64:65, 512:1024])
                rd_dram = dpool.tile([2, 512], F32, tag="rdd")
                nc.sync.dma_start(rd_dram[0:1, :], rd0[:])
                nc.sync.dma_start(rd_dram[1:2, :], rd1[:])
                bc0 = spool.tile([64, 512], F32, tag="bc0")
                bc1 = spool.tile([64, 512], F32, tag="bc1")
                nc.sync.dma_start(bc0[:], rd_dram[0:1, :].to_broadcast((64, 512)))
                nc.sync.dma_start(bc1[:], rd_dram[1:2, :].to_broadcast((64, 512)))
                qs = slice(qb * 512, qb * 512 + 512)
                nc.vector.tensor_mul(ot_c[0:64, hp, qs], osb[0:64, 0:512], bc0[:])
                nc.vector.tensor_mul(ot_c[64:128, hp, qs], osb[0:64, 512:1024], bc1[:])
        while filler:
            filler.pop(0)()

    # prologue: chunk 0 projections
    for cl in proj_closures(0):
        cl()
    for c in range(C):
        filler = []
        pj = proj_closures(c + 1) if c + 1 < C else []
        op = outproj_closures(c - 1) if c >= 1 else []
        # interleave: out-proj groups are ready immediately; proj groups
        # depend on the xt DMA. Alternate so the PE always has ready filler.
        while pj or op:
            if op:
                filler.append(op.pop(0))
            if pj:
                filler.append(pj.pop(0))
        sdpa_chunk(c, filler)
    for cl in outproj_closures(C - 1):
        cl()


def _build(reps: int = 1, stage: str = "full", hw_loop: int = 1):
    nc = bacc.Bacc(
        "TRN2",
        target_bir_lowering=False,
        debug=False,
        enable_asserts=False,
        num_devices=NCORES,
    )
    xt_d = nc.dram_tensor("xt", [D, T], F16, kind="ExternalInput").ap()
    wkq_d = nc.dram_tensor("wkq", [D, 2 * E], F16, kind="ExternalInput").ap()
    wv_d = nc.dram_tensor("wv", [D, E], F16, kind="ExternalInput").ap()
    wo_d = nc.dram_tensor("wo", [E, D], F16, kind="ExternalInput").ap()
    ones_d = nc.dram_tensor("ones", [1, 64], F16, kind="ExternalInput").ap()
    pt_d = nc.dram_tensor("pt", [D, T], F32, kind="ExternalOutput").ap()

    with tile.TileContext(nc) as tc:
        with (
            tc.tile_pool(name="const", bufs=1) as cpool,
            tc.tile_pool(name="xt", bufs=3) as xpool,
            tc.tile_pool(name="kt", bufs=4) as kpool,
            tc.tile_pool(name="qt", bufs=2) as qpool,
            tc.tile_pool(name="v", bufs=4) as vpool,
            tc.tile_pool(name="p", bufs=8) as ppool,
            tc.tile_pool(name="ot", bufs=2) as opool,
            tc.tile_pool(name="sm", bufs=4) as spool,
            tc.tile_pool(name="fin", bufs=4) as fpool,
            tc.tile_pool(name="dram", bufs=4, space="DRAM") as dpool,
            tc.tile_pool(name="ps_s", bufs=2, space="PSUM") as ps_s,
            tc.tile_pool(name="ps_o", bufs=2, space="PSUM") as ps_o,
            tc.tile_pool(name="ps_m", bufs=2, space="PSUM") as ps_m,
        ):
            wkq_sb = cpool.tile([128, 8, 2 * E], F16, tag="wkq")
            nc.sync.dma_start(wkq_sb[:], wkq_d.rearrange("(dt p) e -> p dt e", p=128))
            wv_sb = cpool.tile([128, 8, E], F16, tag="wv")
            nc.sync.dma_start(wv_sb[:], wv_d.rearrange("(dt p) e -> p dt e", p=128))
            wo_sb = cpool.tile([128, 2, D], F16, tag="wo")
            nc.sync.dma_start(wo_sb[:], wo_d.rearrange("(et p) d -> p et d", p=128))
            ones_sb = cpool.tile([1, 64], F16, tag="ones")
            nc.sync.dma_start(ones_sb[:], ones_d)

            pools = (xpool, kpool, qpool, vpool, ppool, opool, spool,
                     fpool, dpool, ps_s, ps_o, ps_m)
            aps = (xt_d, pt_d, wkq_sb, wv_sb, wo_sb, ones_sb)

            if hw_loop > 1:
                with tc.For_i(0, hw_loop, 1):
                    _emit_pass(nc, pools, aps, stage)
            else:
                for _ in range(reps):
                    _emit_pass(nc, pools, aps, stage)

    nc.compile()
    return nc


def _prepare_inputs(x, w_qkv, w_out):
    xt = []
    for b in range(B):
        xt.append(np.ascontiguousarray(x[b].reshape(T, D).T).astype(np.float16))
    ones = np.ones((1, 64), dtype=np.float16)
    in_maps = []
    for core in range(NCORES):
        b, hg = divmod(core, HL)
        e0 = hg * E
        wq = 0.125 * w_qkv[e0 : e0 + E]                 # fold 1/sqrt(HD)
        wk = w_qkv[H * HD + e0 : H * HD + e0 + E]
        wv = w_qkv[2 * H * HD + e0 : 2 * H * HD + e0 + E]
        wkq = np.ascontiguousarray(np.concatenate([wk, wq], axis=0).T).astype(np.float16)
        wv_t = np.ascontiguousarray(wv.T).astype(np.float16)
        wo_t = np.ascontiguousarray(w_out[:, e0 : e0 + E].T).astype(np.float16)
        in_maps.append(
            {"xt": xt[b], "wkq": wkq, "wv": wv_t, "wo": wo_t, "ones": ones}
        )
    return in_maps


def _gather(results):
    out = np.empty((B, F, N, D), dtype=np.float32)
    for b in range(B):
        acc = results[4 * b]["pt"].copy()
        for hg in range(1, HL):
            acc += results[4 * b + hg]["pt"]
        out[b] = acc.T.reshape(F, N, D)
    return out


def run(x, w_qkv, w_out, trace=False, reps=1, stage="full", hw_loop=1):
    key = ("nc", reps, stage, hw_loop)
    if key not in _cached:
        _cached[key] = _build(reps, stage, hw_loop)
    nc = _cached[key]
    in_maps = _prepare_inputs(
        np.asarray(x, dtype=np.float32),
        np.asarray(w_qkv, dtype=np.float32),
        np.asarray(w_out, dtype=np.float32),
    )
    res = run_bass_kernel_spmd(nc, in_maps, core_ids=list(range(NCORES)), trace=trace)
    return _gather(res.results), res


def kernel(x, w_qkv, w_out):
    out, _ = run(x, w_qkv, w_out)
    return out



# revision 6
# speedup vs baseline: 1.5427x; 1.5427x over previous
"""LocalFrameAttention TRN2 kernel.

Problem: x[B=2,F=16,N=256,D=1024] -> qkv proj -> chunked local attention
(chunk = 4 frames = 1024 tokens; chunk c attends to chunks {c-1, c}, chunk 0
to itself) -> out proj.  H=16 heads, HD=64.

Sharding: 8 cores = B(2) x head-groups(4).  Each core handles 4 heads for all
16 frames of one batch: column-parallel qkv projection, full SDPA for its
heads, row-parallel out projection producing a partial [D, T] output; the
host sums the 4 partials per batch (tensor-parallel reduce) and transposes.

Layouts (on-chip activations kept "transposed", d-major):
  - XT [D, T] host-transposed; K^T/Q^T [e=256, T] via lhsT=W^T, rhs=XT;
  - V [T, e] via lhsT=XT, rhs=Wv^T, stored [128, tok_tile, head, 65] with a
    ones 65th column (softmax denominator via the PV matmul);
  - S^T [k_tok=128, q 512 x 2 heads] = mm(lhsT=K^T, rhs=Q^T), K=64
    contraction, two heads row-paired on the PE;
  - P^T = exp(S^T) on ACT (no max subtraction; |logits| small), fp16;
  - O^T accum [65, 512] = mm(lhsT=[V|1], rhs=P^T) over window k-tiles;
  - normalize: O^T copied out of PSUM immediately (frees accumulator banks),
    reciprocal of row 64, partition-broadcast via a DRAM round-trip DMA,
    DVE muls;
  - out proj: partial^T [dd, tok] = mm(lhsT=Wo^T, rhs=O^T).

All matmul operands are fp16 (fp32 PSUM accumulation).  fp16 gets separate,
pipelined LDWEIGHTS (4-byte dtypes force a self-loading matmul that
serializes the weight load) at 2x the mantissa error of fp32r (2^-11).
Next-chunk projection and prev-chunk out-projection matmul groups are
interleaved into the SDPA loop as stall filler (engine instruction order is
static on TRN2).

Measured: ~455 us/core for the full pass (all 8 cores run in parallel),
max relative error vs the fp32 reference ~5e-4.
"""

import sys

if "/opt/trn_rl_repo" not in sys.path:
    sys.path.insert(0, "/opt/trn_rl_repo")

import numpy as np

import concourse.bass as bass  # noqa: F401
import concourse.mybir as mybir
import concourse.tile as tile
from concourse import bacc
from concourse.bass_utils import run_bass_kernel_spmd

F32 = mybir.dt.float32
F32R = mybir.dt.float32r
F16 = mybir.dt.float16
EXP = mybir.ActivationFunctionType.Exp

B, F, N, D = 2, 16, 256, 1024
H, HD, CHUNK = 16, 64, 4
C = F // CHUNK            # 4 chunks
CT = CHUNK * N            # 1024 tokens per chunk
T = F * N                 # 4096 tokens per batch
HL = 4                    # heads per core
E = HL * HD               # 256 local qkv width
NCORES = 8

_cached = {}


def _round_fp32r(a: np.ndarray) -> np.ndarray:
    """Round fp32 array to fp32r (11-bit mantissa, value in high 20 bits)."""
    u = np.ascontiguousarray(a, dtype=np.float32).view(np.uint32)
    r = ((u.astype(np.uint64) + 0x800) & 0xFFFFF000).astype(np.uint32)
    return r.view(np.float32)


def _emit_pass(nc, pools, aps, stage):
    """One full compute pass, software-pipelined: next-chunk projection and
    previous-chunk out-projection matmul groups are interleaved into the SDPA
    kt loop so the PE's static instruction order has independent filler work
    at every exp-dependency stall point."""
    (xpool, kpool, qpool, vpool, ppool, opool, spool, fpool, dpool,
     ps_s, ps_o, ps_m) = pools
    xt_d, pt_d, wkq_sb, wv_sb, wo_sb, ones_sb = aps

    kt_ring = {}
    qt_ring = {}
    v_ring = {}
    xt_tiles = {}
    ot_ring = {}

    def proj_closures(c):
        """19 closures: tile allocs + per-token-block DMA + 8 matmul groups."""
        cl = []

        def alloc(c=c):
            kt_ring[c] = kpool.tile([128, 2, CT], F16, tag="kt", name=f"kt{c}")
            # Q^T per-head slots, zero-padded so QK runs full-array 128x128
            # (lhsT = K head-pair tile; the other head's K rows hit zeros).
            qt_c = qpool.tile([128, 4, CT], F16, tag="qt", name=f"qt{c}")
            for h in range(4):
                z0 = 64 if h % 2 == 0 else 0
                nc.gpsimd.memset(qt_c[z0 : z0 + 64, h, :], 0.0)
            qt_ring[c] = qt_c
            v_c = vpool.tile([128, 8, HL, 68], F16, tag="v")
            nc.gpsimd.memset(v_c[:], 1.0)  # ones col; rest overwritten
            v_ring[c] = v_c

        cl.append(alloc)
        for tb in range(2):
            def dma(c=c, tb=tb):
                xt_t = xpool.tile([128, 8, 512], F16, tag="xt")
                t0 = c * CT + tb * 512
                nc.sync.dma_start(
                    xt_t[:],
                    xt_d[:, t0 : t0 + 512].rearrange("(dt p) t -> p dt t", p=128),
                )
                xt_tiles[(c, tb)] = xt_t

            cl.append(dma)
            for et in range(4):  # K^T (0,1), Q^T (2,3)
                def kq_group(c=c, tb=tb, et=et):
                    xt_t = xt_tiles[(c, tb)]
                    ps = ps_m.tile([128, 512], F32, tag="m")
                    for dt in range(8):
                        nc.tensor.matmul(
                            ps[:],
                            wkq_sb[:, dt, et * 128 : (et + 1) * 128],
                            xt_t[:, dt, :],
                            start=(dt == 0),
                            stop=(dt == 7),
                        )
                    ts = slice(tb * 512, (tb + 1) * 512)
                    if et < 2:
                        nc.vector.tensor_copy(kt_ring[c][:, et, ts], ps[:])
                    else:
                        qt_c = qt_ring[c]
                        h0 = 2 * (et - 2)
                        nc.vector.tensor_copy(qt_c[0:64, h0, ts], ps[0:64, :])
                        nc.vector.tensor_copy(
                            qt_c[64:128, h0 + 1, ts], ps[64:128, :]
                        )

                cl.append(kq_group)
            for tt in range(4):  # V tok-tiles
                def v_group(c=c, tb=tb, tt=tt):
                    xt_t = xt_tiles[(c, tb)]
                    ps = ps_m.tile([128, 512], F32, tag="m")
                    for dt in range(8):
                        nc.tensor.matmul(
                            ps[:, 0:E],
                            xt_t[:, dt, tt * 128 : (tt + 1) * 128],
                            wv_sb[:, dt, :],
                            start=(dt == 0),
                            stop=(dt == 7),
                        )
                    nc.vector.tensor_copy(
                        v_ring[c][:, tb * 4 + tt, :, 0:64],
                        ps[:, 0:E].rearrange("p (h d) -> p h d", h=HL),
                    )

                cl.append(v_group)
        return cl

    def outproj_closures(c):
        cl = []
        for ddt in range(8):
            for tb in range(2):
                def op_group(c=c, ddt=ddt, tb=tb):
                    ot_c = ot_ring[c]
                    fp = ps_m.tile([128, 512], F32, tag="m")
                    for et in range(2):
                        nc.tensor.matmul(
                            fp[:],
                            wo_sb[:, et, ddt * 128 : (ddt + 1) * 128],
                            ot_c[:, et, tb * 512 : (tb + 1) * 512],
                            start=(et == 0),
                            stop=(et == 1),
                        )
                    fin = fpool.tile([128, 512], F16, tag="fin")
                    nc.vector.tensor_copy(fin[:], fp[:])
                    nc.sync.dma_start(
                        pt_d[
                            ddt * 128 : (ddt + 1) * 128,
                            c * CT + tb * 512 : c * CT + (tb + 1) * 512,
                        ],
                        fin[:],
                    )

                cl.append(op_group)
        return cl

    def sdpa_chunk(c, filler):
        """Emit SDPA(c); pop one filler closure after every few kt steps."""
        kts = (
            [(c, i) for i in range(8)]
            if c == 0
            else [(c - 1, i) for i in range(8)] + [(c, i) for i in range(8)]
        )
        ot_ring[c] = opool.tile([128, 2, CT], F16, tag="ot", name=f"ot{c}")
        ot_c = ot_ring[c]
        qt_c = qt_ring[c]

        n_steps = 4 * len(kts)
        stride = max(1, -(-n_steps // max(1, len(filler))) )
        step = 0

        if stage == "proj":
            nc.vector.tensor_copy(ot_c[:, :, 0:4], kt_ring[c][:, :, 0:4])
            while filler:
                filler.pop(0)()
            return

        for hp in range(2):
            for qb in range(2):
                o0 = ps_o.tile([65, 512], F32, tag="o")
                o1 = ps_o.tile([65, 512], F32, tag="o")
                last = len(kts) - 1
                for i, (kc, kt) in enumerate(kts):
                    s = ps_s.tile([128, 1024], F32, tag="s")
                    ktile = kt_ring[kc]
                    for hl in range(2):
                        # full-array 128x128: rhs is the per-head zero-padded
                        # Q slot, so the other head's K rows contribute 0.
                        nc.tensor.matmul(
                            s[:, hl * 512 : hl * 512 + 512],
                            ktile[:, hp, kt * 128 : (kt + 1) * 128],
                            qt_c[:, 2 * hp + hl, qb * 512 : (qb + 1) * 512],
                            start=True,
                            stop=True,
                        )
                    p = ppool.tile([128, 1024], F16, tag="p")
                    if stage in ("qkexp", "full"):
                        nc.scalar.activation(p[:], s[:], EXP)
                    else:
                        nc.vector.tensor_copy(p[:, 0:4], s[:, 0:4])
                    if stage == "full":
                        vt = v_ring[kc]
                        nc.tensor.matmul(
                            o0[:],
                            vt[:, kt, 2 * hp, 0:65],
                            p[:, 0:512],
                            start=(i == 0),
                            stop=(i == last),
                        )
                        nc.tensor.matmul(
                            o1[:],
                            vt[:, kt, 2 * hp + 1, 0:65],
                            p[:, 512:1024],
                            start=(i == 0),
                            stop=(i == last),
                        )
                    elif i == 0:
                        nc.vector.tensor_copy(o0[:, 0:4], p[0:65, 0:4])
                        nc.vector.tensor_copy(o1[:, 0:4], p[0:65, 0:4])
                    step += 1
                    if filler and step % stride == 0:
                        filler.pop(0)()
                # stage O^T out of PSUM immediately (frees the accumulator
                # banks for the next block's PV), then normalize from SBUF
                osb = spool.tile([65, 1024], F32, tag="osb")
                nc.vector.tensor_copy(osb[:, 0:512], o0[:])
                nc.vector.tensor_copy(osb[:, 512:1024], o1[:])
                rd0 = spool.tile([1, 512], F32, tag="rd0")
                rd1 = spool.tile([1, 512], F32, tag="rd1")
                nc.vector.reciprocal(rd0[:], osb[64:65, 0:512])
                nc.vector.reciprocal(rd1[:], osb[64:65, 512:1024])
                rd_dram = dpool.tile([2, 512], F32, tag="rdd")
                nc.sync.dma_start(rd_dram[0:1, :], rd0[:])
                nc.sync.dma_start(rd_dram[1:2, :], rd1[:])
                bc0 = spool.tile([64, 512], F32, tag="bc0")
                bc1 = spool.tile([64, 512], F32, tag="bc1")
                nc.sync.dma_start(bc0[:], rd_dram[0:1, :].to_broadcast((64, 512)))
                nc.sync.dma_start(bc1[:], rd_dram[1:2, :].to_broadcast((64, 512)))
                qs = slice(qb * 512, qb * 512 + 512)
                # partition-aligned mul on Pool; the partition-crossing one
                # stays on DVE (GPSIMD lanes can't shift partitions).
                nc.gpsimd.tensor_mul(ot_c[0:64, hp, qs], osb[0:64, 0:512], bc0[:])
                nc.vector.tensor_mul(ot_c[64:128, hp, qs], osb[0:64, 512:1024], bc1[:])
        while filler:
            filler.pop(0)()

    # prologue: chunk 0 projections
    for cl in proj_closures(0):
        cl()
    for c in range(C):
        filler = []
        pj = proj_closures(c + 1) if c + 1 < C else []
        op = outproj_closures(c - 1) if c >= 1 else []
        # interleave: out-proj groups are ready immediately; proj groups
        # depend on the xt DMA. Alternate so the PE always has ready filler.
        while pj or op:
            if op:
                filler.append(op.pop(0))
            if pj:
                filler.append(pj.pop(0))
        sdpa_chunk(c, filler)
    for cl in outproj_closures(C - 1):
        cl()


def _build(reps: int = 1, stage: str = "full", hw_loop: int = 1):
    nc = bacc.Bacc(
        "TRN2",
        target_bir_lowering=False,
        debug=False,
        enable_asserts=False,
        num_devices=NCORES,
    )
    xt_d = nc.dram_tensor("xt", [D, T], F16, kind="ExternalInput").ap()
    wkq_d = nc.dram_tensor("wkq", [D, 2 * E], F16, kind="ExternalInput").ap()
    wv_d = nc.dram_tensor("wv", [D, E], F16, kind="ExternalInput").ap()
    wo_d = nc.dram_tensor("wo", [E, D], F16, kind="ExternalInput").ap()
    ones_d = nc.dram_tensor("ones", [1, 64], F16, kind="ExternalInput").ap()
    pt_d = nc.dram_tensor("pt", [D, T], F16, kind="ExternalOutput").ap()

    with tile.TileContext(nc) as tc:
        with (
            tc.tile_pool(name="const", bufs=1) as cpool,
            tc.tile_pool(name="xt", bufs=3) as xpool,
            tc.tile_pool(name="kt", bufs=4) as kpool,
            tc.tile_pool(name="qt", bufs=2) as qpool,
            tc.tile_pool(name="v", bufs=4) as vpool,
            tc.tile_pool(name="p", bufs=8) as ppool,
            tc.tile_pool(name="ot", bufs=2) as opool,
            tc.tile_pool(name="sm", bufs=4) as spool,
            tc.tile_pool(name="fin", bufs=4) as fpool,
            tc.tile_pool(name="dram", bufs=4, space="DRAM") as dpool,
            tc.tile_pool(name="ps_s", bufs=2, space="PSUM") as ps_s,
            tc.tile_pool(name="ps_o", bufs=2, space="PSUM") as ps_o,
            tc.tile_pool(name="ps_m", bufs=2, space="PSUM") as ps_m,
        ):
            wkq_sb = cpool.tile([128, 8, 2 * E], F16, tag="wkq")
            nc.sync.dma_start(wkq_sb[:], wkq_d.rearrange("(dt p) e -> p dt e", p=128))
            wv_sb = cpool.tile([128, 8, E], F16, tag="wv")
            nc.sync.dma_start(wv_sb[:], wv_d.rearrange("(dt p) e -> p dt e", p=128))
            wo_sb = cpool.tile([128, 2, D], F16, tag="wo")
            nc.sync.dma_start(wo_sb[:], wo_d.rearrange("(et p) d -> p et d", p=128))
            ones_sb = cpool.tile([1, 64], F16, tag="ones")
            nc.sync.dma_start(ones_sb[:], ones_d)

            pools = (xpool, kpool, qpool, vpool, ppool, opool, spool,
                     fpool, dpool, ps_s, ps_o, ps_m)
            aps = (xt_d, pt_d, wkq_sb, wv_sb, wo_sb, ones_sb)

            if hw_loop > 1:
                with tc.For_i(0, hw_loop, 1):
                    _emit_pass(nc, pools, aps, stage)
            else:
                for _ in range(reps):
                    _emit_pass(nc, pools, aps, stage)

    nc.compile()
    return nc


def _prepare_inputs(x, w_qkv, w_out):
    xt = []
    for b in range(B):
        xt.append(np.ascontiguousarray(x[b].reshape(T, D).T).astype(np.float16))
    ones = np.ones((1, 64), dtype=np.float16)
    in_maps = []
    for core in range(NCORES):
        b, hg = divmod(core, HL)
        e0 = hg * E
        wq = 0.125 * w_qkv[e0 : e0 + E]                 # fold 1/sqrt(HD)
        wk = w_qkv[H * HD + e0 : H * HD + e0 + E]
        wv = w_qkv[2 * H * HD + e0 : 2 * H * HD + e0 + E]
        wkq = np.ascontiguousarray(np.concatenate([wk, wq], axis=0).T).astype(np.float16)
        wv_t = np.ascontiguousarray(wv.T).astype(np.float16)
        wo_t = np.ascontiguousarray(w_out[:, e0 : e0 + E].T).astype(np.float16)
        in_maps.append(
            {"xt": xt[b], "wkq": wkq, "wv": wv_t, "wo": wo_t, "ones": ones}
        )
    return in_maps


def _gather(results):
    out = np.empty((B, F, N, D), dtype=np.float32)
    for b in range(B):
        acc = results[4 * b]["pt"].astype(np.float32)
        for hg in range(1, HL):
            acc += results[4 * b + hg]["pt"].astype(np.float32)
        out[b] = acc.T.reshape(F, N, D)
    return out


def run(x, w_qkv, w_out, trace=False, reps=1, stage="full", hw_loop=1):
    key = ("nc", reps, stage, hw_loop)
    if key not in _cached:
        _cached[key] = _build(reps, stage, hw_loop)
    nc = _cached[key]
    in_maps = _prepare_inputs(
        np.asarray(x, dtype=np.float32),
        np.asarray(w_qkv, dtype=np.float32),
        np.asarray(w_out, dtype=np.float32),
    )
    res = run_bass_kernel_spmd(nc, in_maps, core_ids=list(range(NCORES)), trace=trace)
    return _gather(res.results), res


def kernel(x, w_qkv, w_out):
    out, _ = run(x, w_qkv, w_out)
    return out



# revision 9
# speedup vs baseline: 1.5637x; 1.0137x over previous
"""LocalFrameAttention TRN2 kernel.

Problem: x[B=2,F=16,N=256,D=1024] -> qkv proj -> chunked local attention
(chunk = 4 frames = 1024 tokens; chunk c attends to chunks {c-1, c}, chunk 0
to itself) -> out proj.  H=16 heads, HD=64.

Sharding: 8 cores = B(2) x head-groups(4).  Each core handles 4 heads for all
16 frames of one batch: column-parallel qkv projection, full SDPA for its
heads, row-parallel out projection producing a partial [D, T] output; the
host sums the 4 partials per batch (tensor-parallel reduce) and transposes.

Layouts (on-chip activations kept "transposed", d-major):
  - XT [D, T] host-transposed; K^T/Q^T [e=256, T] via lhsT=W^T, rhs=XT;
  - V [T, e] via lhsT=XT, rhs=Wv^T, stored [128, tok_tile, head, 65] with a
    ones 65th column (softmax denominator via the PV matmul);
  - S^T [k_tok=128, q 512 x 2 heads] = mm(lhsT=K^T, rhs=Q^T), K=64
    contraction, two heads row-paired on the PE;
  - P^T = exp(S^T) on ACT (no max subtraction; |logits| small), fp16;
  - O^T accum [65, 512] = mm(lhsT=[V|1], rhs=P^T) over window k-tiles;
  - normalize: O^T copied out of PSUM immediately (frees accumulator banks),
    reciprocal of row 64, partition-broadcast via a DRAM round-trip DMA,
    DVE muls;
  - out proj: partial^T [dd, tok] = mm(lhsT=Wo^T, rhs=O^T).

All matmul operands are fp16 (fp32 PSUM accumulation).  fp16 gets separate,
pipelined LDWEIGHTS (4-byte dtypes force a self-loading matmul that
serializes the weight load) at 2x the mantissa error of fp32r (2^-11).
Next-chunk projection and prev-chunk out-projection matmul groups are
interleaved into the SDPA loop as stall filler (engine instruction order is
static on TRN2).

Measured: ~455 us/core for the full pass (all 8 cores run in parallel),
max relative error vs the fp32 reference ~5e-4.
"""

import sys

if "/opt/trn_rl_repo" not in sys.path:
    sys.path.insert(0, "/opt/trn_rl_repo")

import numpy as np

import concourse.bass as bass  # noqa: F401
import concourse.mybir as mybir
import concourse.tile as tile
from concourse import bacc
from concourse.bass_utils import run_bass_kernel_spmd

F32 = mybir.dt.float32
F32R = mybir.dt.float32r
F16 = mybir.dt.float16
EXP = mybir.ActivationFunctionType.Exp

B, F, N, D = 2, 16, 256, 1024
H, HD, CHUNK = 16, 64, 4
C = F // CHUNK            # 4 chunks
CT = CHUNK * N            # 1024 tokens per chunk
T = F * N                 # 4096 tokens per batch
HL = 4                    # heads per core
E = HL * HD               # 256 local qkv width
NCORES = 8

_cached = {}


def _round_fp32r(a: np.ndarray) -> np.ndarray:
    """Round fp32 array to fp32r (11-bit mantissa, value in high 20 bits)."""
    u = np.ascontiguousarray(a, dtype=np.float32).view(np.uint32)
    r = ((u.astype(np.uint64) + 0x800) & 0xFFFFF000).astype(np.uint32)
    return r.view(np.float32)


def _emit_pass(nc, pools, aps, stage):
    """One full compute pass, software-pipelined: next-chunk projection and
    previous-chunk out-projection matmul groups are interleaved into the SDPA
    kt loop so the PE's static instruction order has independent filler work
    at every exp-dependency stall point."""
    (xpool, kpool, qpool, vpool, ppool, opool, spool, fpool, dpool,
     ps_s, ps_o, ps_m) = pools
    xt_d, pt_d, wkq_sb, wv_sb, wo_sb, ones_sb = aps

    kt_ring = {}
    qt_ring = {}
    v_ring = {}
    xt_tiles = {}
    ot_ring = {}

    def proj_closures(c):
        """19 closures: tile allocs + per-token-block DMA + 8 matmul groups."""
        cl = []

        def alloc(c=c):
            kt_ring[c] = kpool.tile([128, 2, CT], F16, tag="kt", name=f"kt{c}")
            # Q^T per-head slots, zero-padded so QK runs full-array 128x128
            # (lhsT = K head-pair tile; the other head's K rows hit zeros).
            qt_c = qpool.tile([128, 4, CT], F16, tag="qt", name=f"qt{c}")
            for h in range(4):
                z0 = 64 if h % 2 == 0 else 0
                nc.gpsimd.memset(qt_c[z0 : z0 + 64, h, :], 0.0)
            qt_ring[c] = qt_c
            v_c = vpool.tile([128, 8, HL, 68], F16, tag="v")
            nc.gpsimd.memset(v_c[:], 1.0)  # ones col; rest overwritten
            v_ring[c] = v_c

        cl.append(alloc)
        for tb in range(2):
            def dma(c=c, tb=tb):
                xt_t = xpool.tile([128, 8, 512], F16, tag="xt")
                t0 = c * CT + tb * 512
                nc.sync.dma_start(
                    xt_t[:],
                    xt_d[:, t0 : t0 + 512].rearrange("(dt p) t -> p dt t", p=128),
                )
                xt_tiles[(c, tb)] = xt_t

            cl.append(dma)
            for et in range(4):  # K^T (0,1), Q^T (2,3)
                def kq_group(c=c, tb=tb, et=et):
                    xt_t = xt_tiles[(c, tb)]
                    ps = ps_m.tile([128, 512], F32, tag="m")
                    for dt in range(8):
                        nc.tensor.matmul(
                            ps[:],
                            wkq_sb[:, dt, et * 128 : (et + 1) * 128],
                            xt_t[:, dt, :],
                            start=(dt == 0),
                            stop=(dt == 7),
                        )
                    ts = slice(tb * 512, (tb + 1) * 512)
                    if et < 2:
                        nc.vector.tensor_copy(kt_ring[c][:, et, ts], ps[:])
                    else:
                        qt_c = qt_ring[c]
                        h0 = 2 * (et - 2)
                        nc.vector.tensor_copy(qt_c[0:64, h0, ts], ps[0:64, :])
                        nc.vector.tensor_copy(
                            qt_c[64:128, h0 + 1, ts], ps[64:128, :]
                        )

                cl.append(kq_group)
            for tt in range(4):  # V tok-tiles
                def v_group(c=c, tb=tb, tt=tt):
                    xt_t = xt_tiles[(c, tb)]
                    ps = ps_m.tile([128, 512], F32, tag="m")
                    for dt in range(8):
                        nc.tensor.matmul(
                            ps[:, 0:E],
                            xt_t[:, dt, tt * 128 : (tt + 1) * 128],
                            wv_sb[:, dt, :],
                            start=(dt == 0),
                            stop=(dt == 7),
                        )
                    nc.vector.tensor_copy(
                        v_ring[c][:, tb * 4 + tt, :, 0:64],
                        ps[:, 0:E].rearrange("p (h d) -> p h d", h=HL),
                    )

                cl.append(v_group)
        return cl

    def outproj_closures(c):
        cl = []
        for ddt in range(8):
            for tb in range(2):
                def op_group(c=c, ddt=ddt, tb=tb):
                    ot_c = ot_ring[c]
                    fp = ps_m.tile([128, 512], F32, tag="m")
                    for et in range(2):
                        nc.tensor.matmul(
                            fp[:],
                            wo_sb[:, et, ddt * 128 : (ddt + 1) * 128],
                            ot_c[:, et, tb * 512 : (tb + 1) * 512],
                            start=(et == 0),
                            stop=(et == 1),
                        )
                    fin = fpool.tile([128, 512], F16, tag="fin")
                    nc.vector.tensor_copy(fin[:], fp[:])
                    nc.sync.dma_start(
                        pt_d[
                            ddt * 128 : (ddt + 1) * 128,
                            c * CT + tb * 512 : c * CT + (tb + 1) * 512,
                        ],
                        fin[:],
                    )

                cl.append(op_group)
        return cl

    def sdpa_chunk(c, filler):
        """Emit SDPA(c); pop one filler closure after every few kt steps."""
        kts = (
            [(c, i) for i in range(8)]
            if c == 0
            else [(c - 1, i) for i in range(8)] + [(c, i) for i in range(8)]
        )
        ot_ring[c] = opool.tile([128, 2, CT], F16, tag="ot", name=f"ot{c}")
        ot_c = ot_ring[c]
        qt_c = qt_ring[c]

        n_steps = 4 * len(kts)
        stride = max(1, -(-n_steps // max(1, len(filler))) )
        step = 0

        if stage == "proj":
            nc.vector.tensor_copy(ot_c[:, :, 0:4], kt_ring[c][:, :, 0:4])
            while filler:
                filler.pop(0)()
            return

        for hp in range(2):
            for qb in range(2):
                o0 = ps_o.tile([65, 512], F32, tag="o")
                o1 = ps_o.tile([65, 512], F32, tag="o")
                last = len(kts) - 1
                for i, (kc, kt) in enumerate(kts):
                    s = ps_s.tile([128, 1024], F32, tag="s")
                    ktile = kt_ring[kc]
                    for hl in range(2):
                        # full-array 128x128: rhs is the per-head zero-padded
                        # Q slot, so the other head's K rows contribute 0.
                        nc.tensor.matmul(
                            s[:, hl * 512 : hl * 512 + 512],
                            ktile[:, hp, kt * 128 : (kt + 1) * 128],
                            qt_c[:, 2 * hp + hl, qb * 512 : (qb + 1) * 512],
                            start=True,
                            stop=True,
                        )
                    p = ppool.tile([128, 1024], F16, tag="p")
                    if stage in ("qkexp", "full"):
                        nc.scalar.activation(p[:], s[:], EXP)
                    else:
                        nc.vector.tensor_copy(p[:, 0:4], s[:, 0:4])
                    if stage == "full":
                        vt = v_ring[kc]
                        nc.tensor.matmul(
                            o0[:],
                            vt[:, kt, 2 * hp, 0:65],
                            p[:, 0:512],
                            start=(i == 0),
                            stop=(i == last),
                        )
                        nc.tensor.matmul(
                            o1[:],
                            vt[:, kt, 2 * hp + 1, 0:65],
                            p[:, 512:1024],
                            start=(i == 0),
                            stop=(i == last),
                        )
                    elif i == 0:
                        nc.vector.tensor_copy(o0[:, 0:4], p[0:65, 0:4])
                        nc.vector.tensor_copy(o1[:, 0:4], p[0:65, 0:4])
                    step += 1
                    if filler and step % stride == 0:
                        filler.pop(0)()
                # stage O^T out of PSUM immediately (frees the accumulator
                # banks for the next block's PV), then normalize from SBUF
                osb = spool.tile([65, 1024], F32, tag="osb")
                nc.vector.tensor_copy(osb[:, 0:512], o0[:])
                nc.vector.tensor_copy(osb[:, 512:1024], o1[:])
                rd0 = spool.tile([1, 512], F32, tag="rd0")
                rd1 = spool.tile([1, 512], F32, tag="rd1")
                nc.vector.reciprocal(rd0[:], osb[64:65, 0:512])
                nc.vector.reciprocal(rd1[:], osb[64:65, 512:1024])
                rd_dram = dpool.tile([2, 512], F32, tag="rdd")
                nc.sync.dma_start(rd_dram[0:1, :], rd0[:])
                nc.sync.dma_start(rd_dram[1:2, :], rd1[:])
                bc0 = spool.tile([64, 512], F32, tag="bc0")
                bc1 = spool.tile([64, 512], F32, tag="bc1")
                nc.sync.dma_start(bc0[:], rd_dram[0:1, :].to_broadcast((64, 512)))
                nc.sync.dma_start(bc1[:], rd_dram[1:2, :].to_broadcast((64, 512)))
                qs = slice(qb * 512, qb * 512 + 512)
                # partition-aligned mul on Pool; the partition-crossing one
                # stays on DVE (GPSIMD lanes can't shift partitions).
                nc.gpsimd.tensor_mul(ot_c[0:64, hp, qs], osb[0:64, 0:512], bc0[:])
                nc.vector.tensor_mul(ot_c[64:128, hp, qs], osb[0:64, 512:1024], bc1[:])
        while filler:
            filler.pop(0)()

    # prologue: chunk 0 projections
    for cl in proj_closures(0):
        cl()
    for c in range(C):
        filler = []
        pj = proj_closures(c + 1) if c + 1 < C else []
        op = outproj_closures(c - 1) if c >= 1 else []
        # interleave: out-proj groups are ready immediately; proj groups
        # depend on the xt DMA. Alternate so the PE always has ready filler.
        while pj or op:
            if op:
                filler.append(op.pop(0))
            if pj:
                filler.append(pj.pop(0))
        sdpa_chunk(c, filler)
    for cl in outproj_closures(C - 1):
        cl()


def _build(reps: int = 1, stage: str = "full", hw_loop: int = 1):
    nc = bacc.Bacc(
        "TRN2",
        target_bir_lowering=False,
        debug=False,
        enable_asserts=False,
        num_devices=NCORES,
    )
    xt_d = nc.dram_tensor("xt", [D, T], F16, kind="ExternalInput").ap()
    wkq_d = nc.dram_tensor("wkq", [D, 2 * E], F16, kind="ExternalInput").ap()
    wv_d = nc.dram_tensor("wv", [D, E], F16, kind="ExternalInput").ap()
    wo_d = nc.dram_tensor("wo", [E, D], F16, kind="ExternalInput").ap()
    ones_d = nc.dram_tensor("ones", [1, 64], F16, kind="ExternalInput").ap()
    pt_d = nc.dram_tensor("pt", [D, T], F16, kind="ExternalOutput").ap()

    with tile.TileContext(nc) as tc:
        with (
            tc.tile_pool(name="const", bufs=1) as cpool,
            tc.tile_pool(name="xt", bufs=3) as xpool,
            tc.tile_pool(name="kt", bufs=4) as kpool,
            tc.tile_pool(name="qt", bufs=2) as qpool,
            tc.tile_pool(name="v", bufs=4) as vpool,
            tc.tile_pool(name="p", bufs=8) as ppool,
            tc.tile_pool(name="ot", bufs=2) as opool,
            tc.tile_pool(name="sm", bufs=4) as spool,
            tc.tile_pool(name="fin", bufs=4) as fpool,
            tc.tile_pool(name="dram", bufs=4, space="DRAM") as dpool,
            tc.tile_pool(name="ps_s", bufs=2, space="PSUM") as ps_s,
            tc.tile_pool(name="ps_o", bufs=2, space="PSUM") as ps_o,
            tc.tile_pool(name="ps_m", bufs=2, space="PSUM") as ps_m,
        ):
            wkq_sb = cpool.tile([128, 8, 2 * E], F16, tag="wkq")
            nc.sync.dma_start(wkq_sb[:], wkq_d.rearrange("(dt p) e -> p dt e", p=128))
            wv_sb = cpool.tile([128, 8, E], F16, tag="wv")
            nc.sync.dma_start(wv_sb[:], wv_d.rearrange("(dt p) e -> p dt e", p=128))
            wo_sb = cpool.tile([128, 2, D], F16, tag="wo")
            nc.sync.dma_start(wo_sb[:], wo_d.rearrange("(et p) d -> p et d", p=128))
            ones_sb = cpool.tile([1, 64], F16, tag="ones")
            nc.sync.dma_start(ones_sb[:], ones_d)

            pools = (xpool, kpool, qpool, vpool, ppool, opool, spool,
                     fpool, dpool, ps_s, ps_o, ps_m)
            aps = (xt_d, pt_d, wkq_sb, wv_sb, wo_sb, ones_sb)

            if hw_loop > 1:
                with tc.For_i(0, hw_loop, 1):
                    _emit_pass(nc, pools, aps, stage)
            else:
                for _ in range(reps):
                    _emit_pass(nc, pools, aps, stage)

    nc.compile()
    return nc


def _prepare_inputs(x, w_qkv, w_out):
    xt = []
    for b in range(B):
        xt.append(np.ascontiguousarray(x[b].reshape(T, D).T).astype(np.float16))
    ones = np.ones((1, 64), dtype=np.float16)
    in_maps = []
    for core in range(NCORES):
        b, hg = divmod(core, HL)
        e0 = hg * E
        wq = 0.125 * w_qkv[e0 : e0 + E]                 # fold 1/sqrt(HD)
        wk = w_qkv[H * HD + e0 : H * HD + e0 + E]
        wv = w_qkv[2 * H * HD + e0 : 2 * H * HD + e0 + E]
        wkq = np.ascontiguousarray(np.concatenate([wk, wq], axis=0).T).astype(np.float16)
        wv_t = np.ascontiguousarray(wv.T).astype(np.float16)
        wo_t = np.ascontiguousarray(w_out[:, e0 : e0 + E].T).astype(np.float16)
        in_maps.append(
            {"xt": xt[b], "wkq": wkq, "wv": wv_t, "wo": wo_t, "ones": ones}
        )
    return in_maps


def _gather(results):
    out = np.empty((B, F, N, D), dtype=np.float32)
    for b in range(B):
        acc = results[4 * b]["pt"].astype(np.float32)
        for hg in range(1, HL):
            acc += results[4 * b + hg]["pt"].astype(np.float32)
        out[b] = acc.T.reshape(F, N, D)
    return out


def run(x, w_qkv, w_out, trace=False, reps=1, stage="full", hw_loop=1):
    key = ("nc", reps, stage, hw_loop)
    if key not in _cached:
        _cached[key] = _build(reps, stage, hw_loop)
    nc = _cached[key]
    in_maps = _prepare_inputs(
        np.asarray(x, dtype=np.float32),
        np.asarray(w_qkv, dtype=np.float32),
        np.asarray(w_out, dtype=np.float32),
    )
    res = run_bass_kernel_spmd(nc, in_maps, core_ids=list(range(NCORES)), trace=trace)
    return _gather(res.results), res


def kernel(x, w_qkv, w_out):
    out, _ = run(x, w_qkv, w_out)
    return out

